# revision 4
# baseline (speedup 1.0000x reference)
"""Trainium2 Bass kernel for ErnieLayout self-attention (B=4,S=1024,H=768,NH=12,HD=64).

Sharding: 8 cores = 4 batches x 2 head-groups (6 heads each).
Per-core: QKV projection for its head-group, scores computed TRANSPOSED
([k,q] layout) so rel_pos tiles are PE-transposed (f32r) directly into the
scores PSUM accumulation, the attention mask becomes a per-partition exp
bias, and the softmax denominator falls out of a [V|ones] PV matmul.
Softmax uses exp without max-subtraction (scores are O(10), safe in f32);
masked positions get exp(s-1e10)=0 exactly, matching the reference.
"""
import os
import numpy as np
import ml_dtypes

from concourse import bacc, mybir, tile
from concourse.bass_utils import run_bass_kernel_spmd
from concourse.masks import make_identity

B, S, H = 4, 1024, 768
NH, HD = 12, 64
N_CORES = 8
HPC = 6            # heads per core
COLS = HPC * HD    # 384 output columns per core
KC = H // 128      # 6 contraction chunks for projections
SC = S // 128      # 8 S chunks
QH = 2             # q halves of 512
bf16 = mybir.dt.bfloat16
f32 = mybir.dt.float32
f32r = mybir.dt.float32r
i32 = mybir.dt.int32
AF = mybir.ActivationFunctionType
BF16_NP = ml_dtypes.bfloat16

_compiled = None
last_result = None  # BassKernelResults of the most recent run (for test harness)


def _build():
    nc = bacc.Bacc("TRN2", target_bir_lowering=False, debug=False,
                   num_devices=N_CORES)
    hs = nc.dram_tensor("hs", [S, H], bf16, kind="ExternalInput").ap()
    wq = nc.dram_tensor("wq", [H, COLS], bf16, kind="ExternalInput").ap()
    wk = nc.dram_tensor("wk", [H, COLS], bf16, kind="ExternalInput").ap()
    wv = nc.dram_tensor("wv", [H, COLS], bf16, kind="ExternalInput").ap()
    bq = nc.dram_tensor("bq", [COLS], f32, kind="ExternalInput").ap()
    bk = nc.dram_tensor("bk", [COLS], f32, kind="ExternalInput").ap()
    bv = nc.dram_tensor("bv", [COLS], f32, kind="ExternalInput").ap()
    rel1 = nc.dram_tensor("rel1", [HPC, S, S], bf16, kind="ExternalInput").ap()
    rel2 = nc.dram_tensor("rel2", [HPC, S, S], bf16, kind="ExternalInput").ap()
    mask = nc.dram_tensor("mask", [S], i32, kind="ExternalInput").ap()
    out = nc.dram_tensor("out", [S, COLS], f32, kind="ExternalOutput").ap()

    with tile.TileContext(nc) as tc:
        with tc.tile_pool(name="const", bufs=1) as const, \
             tc.tile_pool(name="hst", bufs=1) as hst_pool, \
             tc.tile_pool(name="w", bufs=1) as w_pool, \
             tc.tile_pool(name="qk", bufs=1) as qk_pool, \
             tc.tile_pool(name="v", bufs=1) as v_pool, \
             tc.tile_pool(name="r1", bufs=2) as r1_pool, \
             tc.tile_pool(name="r2", bufs=2) as r2_pool, \
             tc.tile_pool(name="r12", bufs=2) as r12_pool, \
             tc.tile_pool(name="et", bufs=2) as e_pool, \
             tc.tile_pool(name="ctxt", bufs=2) as ctxt_pool, \
             tc.tile_pool(name="ob", bufs=4) as ob_pool, \
             tc.tile_pool(name="psA", bufs=2, space="PSUM") as proj_psum, \
             tc.tile_pool(name="psS", bufs=3, space="PSUM") as sc_psum, \
             tc.tile_pool(name="psV", bufs=2, space="PSUM") as pv_psum, \
             tc.tile_pool(name="psT", bufs=1, space="PSUM") as pt_psum:

            # ---- constants ----
            ident_f32 = const.tile([128, 128], f32)
            make_identity(nc, ident_f32)
            ident_r = const.tile([128, 128], f32r)
            nc.vector.tensor_copy(ident_r, ident_f32)

            mask_i = const.tile([128, SC], i32)
            nc.sync.dma_start(out=mask_i, in_=mask.rearrange("(c p) -> p c", p=128))
            maskb = const.tile([128, SC], f32)
            nc.vector.tensor_copy(maskb, mask_i)
            nc.vector.tensor_scalar_mul(maskb, maskb, -1e10)

            bq_sb = const.tile([128, 3], f32)
            nc.sync.dma_start(out=bq_sb, in_=bq.rearrange("(c p) -> p c", p=128))
            bk_sb = const.tile([128, 3], f32)
            nc.sync.dma_start(out=bk_sb, in_=bk.rearrange("(c p) -> p c", p=128))
            import concourse.bass as bass
            bv_bc = bass.AP(tensor=bv.tensor, offset=bv.offset,
                            ap=[[0, 128]] + list(bv.ap))
            bv_sb = const.tile([128, COLS], f32)
            nc.gpsimd.dma_start(out=bv_sb, in_=bv_bc)

            # ---- hsT: [H, S] via DMA transpose ----
            hsT = hst_pool.tile([128, KC, S], bf16)
            for k in range(KC):
                nc.sync.dma_start(out=hsT[:, k, :], in_=hs[:, k * 128:(k + 1) * 128],
                                  transpose=True)

            # ---- weights ----
            wq_sb = w_pool.tile([128, KC, COLS], bf16)
            nc.sync.dma_start(out=wq_sb, in_=wq.rearrange("(c p) n -> p c n", p=128))
            wk_sb = w_pool.tile([128, KC, COLS], bf16)
            nc.sync.dma_start(out=wk_sb, in_=wk.rearrange("(c p) n -> p c n", p=128))
            wv_sb = w_pool.tile([128, KC, COLS], bf16)
            nc.sync.dma_start(out=wv_sb, in_=wv.rearrange("(c p) n -> p c n", p=128))

            # ---- projections ----
            # qT/kT: [d(2 heads stacked), S] per head-pair hp; q scaled by 1/8
            qT = qk_pool.tile([128, 3, S], bf16)
            kT = qk_pool.tile([128, 3, S], bf16)
            for hp in range(3):
                for sh in range(QH):
                    ssl = slice(sh * 512, (sh + 1) * 512)
                    psq = proj_psum.tile([128, 512], f32, tag="proj")
                    for k in range(KC):
                        nc.tensor.matmul(psq, wq_sb[:, k, hp * 128:(hp + 1) * 128],
                                         hsT[:, k, ssl],
                                         start=(k == 0), stop=(k == KC - 1))
                    nc.scalar.activation(out=qT[:, hp, ssl], in_=psq, func=AF.Identity,
                                         bias=bq_sb[:, hp:hp + 1], scale=0.125)
                    psk = proj_psum.tile([128, 512], f32, tag="proj")
                    for k in range(KC):
                        nc.tensor.matmul(psk, wk_sb[:, k, hp * 128:(hp + 1) * 128],
                                         hsT[:, k, ssl],
                                         start=(k == 0), stop=(k == KC - 1))
                    nc.scalar.activation(out=kT[:, hp, ssl], in_=psk, func=AF.Identity,
                                         bias=bk_sb[:, hp:hp + 1], scale=1.0)

            # v: [S, head, d] layout + ones column (col 64) for denominators
            v_sb = v_pool.tile([128, SC, HPC, HD + 1], bf16)
            nc.vector.memset(v_sb[:, :, :, HD], 1.0)
            for sc in range(SC):
                psv_full = proj_psum.tile([128, 512], f32, tag="proj")
                psv = psv_full[:, 0:384]
                for k in range(KC):
                    nc.tensor.matmul(psv, hsT[:, k, sc * 128:(sc + 1) * 128],
                                     wv_sb[:, k, :],
                                     start=(k == 0), stop=(k == KC - 1))
                nc.vector.tensor_add(
                    v_sb[:, sc, :, 0:HD],
                    psv.rearrange("p (h d) -> p h d", h=HPC),
                    bv_sb.rearrange("p (h d) -> p h d", h=HPC))

            # ---- attention ----
            for h in range(HPC):
                hp, hi = divmod(h, 2)
                dsl = slice(hi * 64, (hi + 1) * 64)
                for qh in range(QH):
                    qsl = slice(qh * 512, (qh + 1) * 512)
                    r1 = r1_pool.tile([128, 4, S], bf16)
                    nc.sync.dma_start(
                        out=r1, in_=rel1[h, qh * 512:(qh + 1) * 512, :]
                        .rearrange("(i p) k -> p i k", p=128))
                    r2 = r2_pool.tile([128, 4, S], bf16)
                    nc.sync.dma_start(
                        out=r2, in_=rel2[h, qh * 512:(qh + 1) * 512, :]
                        .rearrange("(i p) k -> p i k", p=128))
                    r12 = r12_pool.tile([128, 4, S], f32r)
                    nc.vector.tensor_add(r12, r1, r2)

                    et = e_pool.tile([128, SC, 512], bf16)
                    for kc in range(SC):
                        ksl = slice(kc * 128, (kc + 1) * 128)
                        ps = sc_psum.tile([128, 512], f32)
                        # rel12[q,k].T accumulated into scoresT psum
                        for i in range(4):
                            nc.tensor.matmul(
                                ps[:, i * 128:(i + 1) * 128].bitcast(f32r),
                                r12[:, i, ksl], ident_r,
                                is_transpose=True, start=(i == 0), stop=False)
                        # += k^T q (K=64 contraction over head dim)
                        nc.tensor.matmul(ps, kT[dsl, hp, ksl], qT[dsl, hp, qsl],
                                         start=False, stop=True)
                        nc.scalar.activation(out=et[:, kc, :], in_=ps, func=AF.Exp,
                                             bias=maskb[:, kc:kc + 1], scale=1.0)

                    pv = pv_psum.tile([HD + 1, 512], f32)
                    for kc in range(SC):
                        nc.tensor.matmul(pv, v_sb[:, kc, h, :], et[:, kc, :],
                                         start=(kc == 0), stop=(kc == SC - 1))
                    ctxT = ctxt_pool.tile([HD + 1, 512], f32)
                    nc.vector.tensor_copy(ctxT, pv)
                    for i in range(4):
                        pt = pt_psum.tile([128, HD + 1], f32)
                        nc.tensor.matmul(pt, ctxT[:, i * 128:(i + 1) * 128],
                                         ident_f32[:HD + 1, :HD + 1],
                                         is_transpose=True, start=True, stop=True)
                        rec = ob_pool.tile([128, 1], f32, tag="rec")
                        nc.vector.reciprocal(rec, pt[:, HD:HD + 1])
                        ob = ob_pool.tile([128, HD], f32, tag="ob")
                        nc.vector.tensor_scalar_mul(ob, pt[:, 0:HD], rec)
                        nc.sync.dma_start(
                            out=out[qh * 512 + i * 128: qh * 512 + (i + 1) * 128,
                                    h * HD:(h + 1) * HD],
                            in_=ob)

    nc.compile()
    return nc


def _get_compiled():
    global _compiled
    if _compiled is None:
        _compiled = _build()
    return _compiled


def kernel(hidden_states, Wq, bq, Wk, bk, Wv, bv, rel_pos, rel_2d_pos,
           attention_mask, _trace=False):
    global last_result
    nc = _get_compiled()

    hidden_states = np.asarray(hidden_states, np.float32)
    Wq, Wk, Wv = (np.asarray(w, np.float32) for w in (Wq, Wk, Wv))
    bq, bk, bv = (np.asarray(x, np.float32) for x in (bq, bk, bv))
    rel_pos = np.asarray(rel_pos, np.float32)
    rel_2d_pos = np.asarray(rel_2d_pos, np.float32)
    attention_mask = np.asarray(attention_mask, np.int32)

    in_maps = []
    for c in range(N_CORES):
        b, hg = divmod(c, 2)
        cs = slice(hg * COLS, (hg + 1) * COLS)
        h0 = hg * HPC
        in_maps.append({
            "hs": hidden_states[b].astype(BF16_NP),
            "wq": Wq[:, cs].astype(BF16_NP),
            "wk": Wk[:, cs].astype(BF16_NP),
            "wv": Wv[:, cs].astype(BF16_NP),
            "bq": np.ascontiguousarray(bq[cs]) * np.float32(0.125),
            "bk": np.ascontiguousarray(bk[cs]),
            "bv": np.ascontiguousarray(bv[cs]),
            "rel1": rel_pos[b, h0:h0 + HPC].astype(BF16_NP),
            "rel2": rel_2d_pos[b, h0:h0 + HPC].astype(BF16_NP),
            "mask": np.ascontiguousarray(attention_mask[b, 0, 0]),
        })

    kwargs = {}
    if _trace or os.environ.get("KERNEL_TRACE"):
        kwargs["trace"] = True
    last_result = run_bass_kernel_spmd(nc, in_maps, list(range(N_CORES)), **kwargs)

    result = np.empty((B, S, H), np.float32)
    for c in range(N_CORES):
        b, hg = divmod(c, 2)
        result[b, :, hg * COLS:(hg + 1) * COLS] = last_result.results[c]["out"]
    return result


# revision 11
# speedup vs baseline: 1.0591x; 1.0591x over previous
"""Trainium2 Bass kernel for ErnieLayout self-attention (B=4,S=1024,H=768,NH=12,HD=64).

Sharding: 8 cores = 4 batches x 2 head-groups (6 heads each).
Per-core: QKV projection for its head-group, scores computed TRANSPOSED
([k,q] layout) so rel_pos tiles are PE-transposed (f32r) directly into the
scores PSUM accumulation, the attention mask becomes a per-partition exp
bias, and the softmax denominator falls out of a [V|ones] PV matmul.
Softmax uses exp without max-subtraction (scores are O(10), safe in f32);
masked positions get exp(s-1e10)=0 exactly, matching the reference.
"""
import os
import numpy as np
import ml_dtypes

from concourse import bacc, mybir, tile
from concourse.bass_utils import run_bass_kernel_spmd
from concourse.masks import make_identity

B, S, H = 4, 1024, 768
NH, HD = 12, 64
N_CORES = 8
HPC = 6            # heads per core
COLS = HPC * HD    # 384 output columns per core
KC = H // 128      # 6 contraction chunks for projections
SC = S // 128      # 8 S chunks
QH = 2             # q halves of 512
bf16 = mybir.dt.bfloat16
f32 = mybir.dt.float32
f32r = mybir.dt.float32r
i32 = mybir.dt.int32
AF = mybir.ActivationFunctionType
BF16_NP = ml_dtypes.bfloat16

_compiled = None
last_result = None  # BassKernelResults of the most recent run (for test harness)


def _build():
    nc = bacc.Bacc("TRN2", target_bir_lowering=False, debug=False,
                   num_devices=N_CORES)
    hs = nc.dram_tensor("hs", [S, H], bf16, kind="ExternalInput").ap()
    wq = nc.dram_tensor("wq", [H, COLS], bf16, kind="ExternalInput").ap()
    wk = nc.dram_tensor("wk", [H, COLS], bf16, kind="ExternalInput").ap()
    wv = nc.dram_tensor("wv", [H, COLS], bf16, kind="ExternalInput").ap()
    bq = nc.dram_tensor("bq", [COLS], f32, kind="ExternalInput").ap()
    bk = nc.dram_tensor("bk", [COLS], f32, kind="ExternalInput").ap()
    bv = nc.dram_tensor("bv", [COLS], f32, kind="ExternalInput").ap()
    rel1 = nc.dram_tensor("rel1", [HPC, S, S], bf16, kind="ExternalInput").ap()
    rel2 = nc.dram_tensor("rel2", [HPC, S, S], bf16, kind="ExternalInput").ap()
    mask = nc.dram_tensor("mask", [S], i32, kind="ExternalInput").ap()
    out = nc.dram_tensor("out", [S, COLS], f32, kind="ExternalOutput").ap()

    with tile.TileContext(nc) as tc:
        with tc.tile_pool(name="const", bufs=1) as const, \
             tc.tile_pool(name="hst", bufs=1) as hst_pool, \
             tc.tile_pool(name="w", bufs=1) as w_pool, \
             tc.tile_pool(name="qk", bufs=1) as qk_pool, \
             tc.tile_pool(name="v", bufs=1) as v_pool, \
             tc.tile_pool(name="r1", bufs=2) as r1_pool, \
             tc.tile_pool(name="r2", bufs=2) as r2_pool, \
             tc.tile_pool(name="r12", bufs=2) as r12_pool, \
             tc.tile_pool(name="et", bufs=2) as e_pool, \
             tc.tile_pool(name="ctxt", bufs=2) as ctxt_pool, \
             tc.tile_pool(name="ob", bufs=4) as ob_pool:

            # ---- constants ----
            ident_f32 = const.tile([128, 128], f32)
            make_identity(nc, ident_f32)
            ident_r = const.tile([128, 128], f32r)
            nc.vector.tensor_copy(ident_r, ident_f32)

            mask_i = const.tile([128, SC], i32)
            nc.sync.dma_start(out=mask_i, in_=mask.rearrange("(c p) -> p c", p=128))
            maskb = const.tile([128, SC], f32)
            nc.vector.tensor_copy(maskb, mask_i)
            nc.vector.tensor_scalar_mul(maskb, maskb, -1e10)

            bq_sb = const.tile([128, 3], f32)
            nc.sync.dma_start(out=bq_sb, in_=bq.rearrange("(c p) -> p c", p=128))
            bk_sb = const.tile([128, 3], f32)
            nc.sync.dma_start(out=bk_sb, in_=bk.rearrange("(c p) -> p c", p=128))
            import concourse.bass as bass
            bv_bc = bass.AP(tensor=bv.tensor, offset=bv.offset,
                            ap=[[0, 128]] + list(bv.ap))
            bv_sb = const.tile([128, COLS], f32)
            nc.gpsimd.dma_start(out=bv_sb, in_=bv_bc)

            # ---- hsT: [H, S] via DMA transpose ----
            hsT = hst_pool.tile([128, KC, S], bf16)
            for k in range(KC):
                nc.sync.dma_start(out=hsT[:, k, :], in_=hs[:, k * 128:(k + 1) * 128],
                                  transpose=True)

            # ---- weights ----
            wq_sb = w_pool.tile([128, KC, COLS], bf16)
            nc.sync.dma_start(out=wq_sb, in_=wq.rearrange("(c p) n -> p c n", p=128))
            wk_sb = w_pool.tile([128, KC, COLS], bf16)
            nc.sync.dma_start(out=wk_sb, in_=wk.rearrange("(c p) n -> p c n", p=128))
            wv_sb = w_pool.tile([128, KC, COLS], bf16)
            nc.sync.dma_start(out=wv_sb, in_=wv.rearrange("(c p) n -> p c n", p=128))

            proj_ctx = tc.tile_pool(name="psA", bufs=2, space="PSUM")
            proj_psum = proj_ctx.__enter__()
            # ---- projections ----
            # qT: [d(2 heads stacked), S] per head-pair hp; q scaled by 1/8.
            # kT zero-padded per head to K=128 (kTz[:, hp, hi]: head hi's 64
            # d-rows live at their stacked position, other 64 rows are 0) so
            # the scores matmul streams a full-width 128-partition rhs.
            qT = qk_pool.tile([128, 3, S], bf16)
            kTz = qk_pool.tile([128, 3, 2, S], bf16)
            nc.vector.memset(kTz, 0.0)
            for hp in range(3):
                for sh in range(QH):
                    ssl = slice(sh * 512, (sh + 1) * 512)
                    psq = proj_psum.tile([128, 512], f32, tag="proj")
                    for k in range(KC):
                        nc.tensor.matmul(psq, wq_sb[:, k, hp * 128:(hp + 1) * 128],
                                         hsT[:, k, ssl],
                                         start=(k == 0), stop=(k == KC - 1))
                    nc.scalar.activation(out=qT[:, hp, ssl], in_=psq, func=AF.Identity,
                                         bias=bq_sb[:, hp:hp + 1], scale=0.125)
                    psk = proj_psum.tile([128, 512], f32, tag="proj")
                    for k in range(KC):
                        nc.tensor.matmul(psk, wk_sb[:, k, hp * 128:(hp + 1) * 128],
                                         hsT[:, k, ssl],
                                         start=(k == 0), stop=(k == KC - 1))
                    nc.scalar.activation(out=kTz[0:64, hp, 0, ssl], in_=psk[0:64, :],
                                         func=AF.Identity,
                                         bias=bk_sb[0:64, hp:hp + 1], scale=1.0)
                    nc.scalar.activation(out=kTz[64:128, hp, 1, ssl],
                                         in_=psk[64:128, :], func=AF.Identity,
                                         bias=bk_sb[64:128, hp:hp + 1], scale=1.0)

            # v: [S, head, d] layout + ones column (col 64) for denominators
            v_sb = v_pool.tile([128, SC, HPC, HD + 1], bf16)
            nc.vector.memset(v_sb[:, :, :, HD], 1.0)
            for sc in range(SC):
                psv_full = proj_psum.tile([128, 512], f32, tag="proj")
                psv = psv_full[:, 0:384]
                for k in range(KC):
                    nc.tensor.matmul(psv, hsT[:, k, sc * 128:(sc + 1) * 128],
                                     wv_sb[:, k, :],
                                     start=(k == 0), stop=(k == KC - 1))
                nc.vector.tensor_add(
                    v_sb[:, sc, :, 0:HD],
                    psv.rearrange("p (h d) -> p h d", h=HPC),
                    bv_sb.rearrange("p (h d) -> p h d", h=HPC))
            proj_ctx.__exit__(None, None, None)

            # ---- attention ----
            att_ctx = [tc.tile_pool(name="psS", bufs=4, space="PSUM"),
                       tc.tile_pool(name="psV", bufs=2, space="PSUM"),
                       tc.tile_pool(name="psT", bufs=1, space="PSUM")]
            sc_psum, pv_psum, pt_psum = [c.__enter__() for c in att_ctx]
            for h in range(HPC):
                hp, hi = divmod(h, 2)
                dsl = slice(hi * 64, (hi + 1) * 64)
                for qh in range(QH):
                    qsl = slice(qh * 512, (qh + 1) * 512)
                    r1 = r1_pool.tile([128, 4, S], bf16)
                    nc.sync.dma_start(
                        out=r1, in_=rel1[h, qh * 512:(qh + 1) * 512, :]
                        .rearrange("(i p) k -> p i k", p=128))
                    r2 = r2_pool.tile([128, 4, S], bf16)
                    nc.sync.dma_start(
                        out=r2, in_=rel2[h, qh * 512:(qh + 1) * 512, :]
                        .rearrange("(i p) k -> p i k", p=128))
                    r12 = r12_pool.tile([128, 4, S], f32r)
                    nc.vector.tensor_add(r12, r1, r2)

                    et = e_pool.tile([128, SC, 512], bf16)
                    for kc in range(SC):
                        ksl = slice(kc * 128, (kc + 1) * 128)
                        ps = sc_psum.tile([128, 512], f32)
                        # rel12[q,k].T accumulated into scoresT psum
                        for i in range(4):
                            nc.tensor.matmul(
                                ps[:, i * 128:(i + 1) * 128].bitcast(f32r),
                                r12[:, i, ksl], ident_r,
                                is_transpose=True, start=(i == 0), stop=False)
                        # += k^T q (zero-padded K=128: head hi's rows live at
                        # dsl, the other head's qT rows hit zeros in kTz)
                        nc.tensor.matmul(ps, kTz[:, hp, hi, ksl], qT[:, hp, qsl],
                                         start=False, stop=True)
                        nc.scalar.activation(out=et[:, kc, :], in_=ps, func=AF.Exp,
                                             bias=maskb[:, kc:kc + 1], scale=1.0)

                    pv = pv_psum.tile([HD + 1, 512], f32)
                    for kc in range(SC):
                        nc.tensor.matmul(pv, v_sb[:, kc, h, :], et[:, kc, :],
                                         start=(kc == 0), stop=(kc == SC - 1))
                    ctxT = ctxt_pool.tile([HD + 1, 512], f32)
                    nc.vector.tensor_copy(ctxT, pv)
                    for i in range(4):
                        pt = pt_psum.tile([128, HD + 1], f32)
                        nc.tensor.matmul(pt, ctxT[:, i * 128:(i + 1) * 128],
                                         ident_f32[:HD + 1, :HD + 1],
                                         is_transpose=True, start=True, stop=True)
                        rec = ob_pool.tile([128, 1], f32, tag="rec")
                        nc.vector.reciprocal(rec, pt[:, HD:HD + 1])
                        ob = ob_pool.tile([128, HD], f32, tag="ob")
                        nc.vector.tensor_scalar_mul(ob, pt[:, 0:HD], rec)
                        nc.sync.dma_start(
                            out=out[qh * 512 + i * 128: qh * 512 + (i + 1) * 128,
                                    h * HD:(h + 1) * HD],
                            in_=ob)
            for c in reversed(att_ctx):
                c.__exit__(None, None, None)

    nc.compile()
    return nc


def _get_compiled():
    global _compiled
    if _compiled is None:
        _compiled = _build()
    return _compiled


def kernel(hidden_states, Wq, bq, Wk, bk, Wv, bv, rel_pos, rel_2d_pos,
           attention_mask, _trace=False):
    global last_result
    nc = _get_compiled()

    hidden_states = np.asarray(hidden_states, np.float32)
    Wq, Wk, Wv = (np.asarray(w, np.float32) for w in (Wq, Wk, Wv))
    bq, bk, bv = (np.asarray(x, np.float32) for x in (bq, bk, bv))
    rel_pos = np.asarray(rel_pos, np.float32)
    rel_2d_pos = np.asarray(rel_2d_pos, np.float32)
    attention_mask = np.asarray(attention_mask, np.int32)

    in_maps = []
    for c in range(N_CORES):
        b, hg = divmod(c, 2)
        cs = slice(hg * COLS, (hg + 1) * COLS)
        h0 = hg * HPC
        in_maps.append({
            "hs": hidden_states[b].astype(BF16_NP),
            "wq": Wq[:, cs].astype(BF16_NP),
            "wk": Wk[:, cs].astype(BF16_NP),
            "wv": Wv[:, cs].astype(BF16_NP),
            "bq": np.ascontiguousarray(bq[cs]) * np.float32(0.125),
            "bk": np.ascontiguousarray(bk[cs]),
            "bv": np.ascontiguousarray(bv[cs]),
            "rel1": rel_pos[b, h0:h0 + HPC].astype(BF16_NP),
            "rel2": rel_2d_pos[b, h0:h0 + HPC].astype(BF16_NP),
            "mask": np.ascontiguousarray(attention_mask[b, 0, 0]),
        })

    kwargs = {}
    if _trace or os.environ.get("KERNEL_TRACE"):
        kwargs["trace"] = True
    last_result = run_bass_kernel_spmd(nc, in_maps, list(range(N_CORES)), **kwargs)

    result = np.empty((B, S, H), np.float32)
    for c in range(N_CORES):
        b, hg = divmod(c, 2)
        result[b, :, hg * COLS:(hg + 1) * COLS] = last_result.results[c]["out"]
    return result


# revision 12
# speedup vs baseline: 1.0964x; 1.0352x over previous
"""Trainium2 Bass kernel for ErnieLayout self-attention (B=4,S=1024,H=768,NH=12,HD=64).

Sharding: 8 cores = 4 batches x 2 head-groups (6 heads each).
Per-core: QKV projection for its head-group, scores computed TRANSPOSED
([k,q] layout) so rel_pos tiles are PE-transposed (f32r) directly into the
scores PSUM accumulation, the attention mask becomes a per-partition exp
bias, and the softmax denominator falls out of a [V|ones] PV matmul.
Softmax uses exp without max-subtraction (scores are O(10), safe in f32);
masked positions get exp(s-1e10)=0 exactly, matching the reference.
"""
import os
import numpy as np
import ml_dtypes

from concourse import bacc, mybir, tile
from concourse.bass_utils import run_bass_kernel_spmd
from concourse.masks import make_identity

B, S, H = 4, 1024, 768
NH, HD = 12, 64
N_CORES = 8
HPC = 6            # heads per core
COLS = HPC * HD    # 384 output columns per core
KC = H // 128      # 6 contraction chunks for projections
SC = S // 128      # 8 S chunks
QH = 2             # q halves of 512
bf16 = mybir.dt.bfloat16
f32 = mybir.dt.float32
f32r = mybir.dt.float32r
i32 = mybir.dt.int32
AF = mybir.ActivationFunctionType
BF16_NP = ml_dtypes.bfloat16

_compiled = None
last_result = None  # BassKernelResults of the most recent run (for test harness)


def _build():
    nc = bacc.Bacc("TRN2", target_bir_lowering=False, debug=False,
                   num_devices=N_CORES)
    hs = nc.dram_tensor("hs", [S, H], bf16, kind="ExternalInput").ap()
    wq = nc.dram_tensor("wq", [H, COLS], bf16, kind="ExternalInput").ap()
    wk = nc.dram_tensor("wk", [H, COLS], bf16, kind="ExternalInput").ap()
    wv = nc.dram_tensor("wv", [H, COLS], bf16, kind="ExternalInput").ap()
    bq = nc.dram_tensor("bq", [COLS], f32, kind="ExternalInput").ap()
    bk = nc.dram_tensor("bk", [COLS], f32, kind="ExternalInput").ap()
    bv = nc.dram_tensor("bv", [COLS], f32, kind="ExternalInput").ap()
    rel1 = nc.dram_tensor("rel1", [HPC, S, S], bf16, kind="ExternalInput").ap()
    rel2 = nc.dram_tensor("rel2", [HPC, S, S], bf16, kind="ExternalInput").ap()
    mask = nc.dram_tensor("mask", [S], i32, kind="ExternalInput").ap()
    out = nc.dram_tensor("out", [S, COLS], f32, kind="ExternalOutput").ap()

    with tile.TileContext(nc) as tc:
        with tc.tile_pool(name="const", bufs=1) as const, \
             tc.tile_pool(name="hst", bufs=1) as hst_pool, \
             tc.tile_pool(name="w", bufs=1) as w_pool, \
             tc.tile_pool(name="qk", bufs=1) as qk_pool, \
             tc.tile_pool(name="v", bufs=1) as v_pool, \
             tc.tile_pool(name="r1", bufs=2) as r1_pool, \
             tc.tile_pool(name="r2", bufs=2) as r2_pool, \
             tc.tile_pool(name="r12", bufs=2) as r12_pool, \
             tc.tile_pool(name="et", bufs=16) as e_pool, \
             tc.tile_pool(name="ctxt", bufs=2) as ctxt_pool, \
             tc.tile_pool(name="ob", bufs=4) as ob_pool:

            # ---- constants ----
            ident_f32 = const.tile([128, 128], f32)
            make_identity(nc, ident_f32)
            ident_r = const.tile([128, 128], f32r)
            nc.vector.tensor_copy(ident_r, ident_f32)

            mask_i = const.tile([128, SC], i32)
            nc.sync.dma_start(out=mask_i, in_=mask.rearrange("(c p) -> p c", p=128))
            maskb = const.tile([128, SC], f32)
            nc.vector.tensor_copy(maskb, mask_i)
            nc.vector.tensor_scalar_mul(maskb, maskb, -1e10)

            bq_sb = const.tile([128, 3], f32)
            nc.sync.dma_start(out=bq_sb, in_=bq.rearrange("(c p) -> p c", p=128))
            bk_sb = const.tile([128, 3], f32)
            nc.sync.dma_start(out=bk_sb, in_=bk.rearrange("(c p) -> p c", p=128))
            import concourse.bass as bass
            bv_bc = bass.AP(tensor=bv.tensor, offset=bv.offset,
                            ap=[[0, 128]] + list(bv.ap))
            bv_sb = const.tile([128, COLS], f32)
            nc.gpsimd.dma_start(out=bv_sb, in_=bv_bc)

            # ---- hsT: [H, S] via DMA transpose ----
            hsT = hst_pool.tile([128, KC, S], bf16)
            for k in range(KC):
                nc.sync.dma_start(out=hsT[:, k, :], in_=hs[:, k * 128:(k + 1) * 128],
                                  transpose=True)

            # ---- weights ----
            wq_sb = w_pool.tile([128, KC, COLS], bf16)
            for k in range(KC):
                nc.sync.dma_start(out=wq_sb[:, k, :],
                                  in_=wq[k * 128:(k + 1) * 128, :])
            wk_sb = w_pool.tile([128, KC, COLS], bf16)
            for k in range(KC):
                nc.sync.dma_start(out=wk_sb[:, k, :],
                                  in_=wk[k * 128:(k + 1) * 128, :])
            wv_sb = w_pool.tile([128, KC, COLS], bf16)
            for k in range(KC):
                nc.sync.dma_start(out=wv_sb[:, k, :],
                                  in_=wv[k * 128:(k + 1) * 128, :])

            proj_ctx = tc.tile_pool(name="psA", bufs=2, space="PSUM")
            proj_psum = proj_ctx.__enter__()
            # ---- projections ----
            # qT: [d(2 heads stacked), S] per head-pair hp; q scaled by 1/8.
            # kT zero-padded per head to K=128 (kTz[:, hp, hi]: head hi's 64
            # d-rows live at their stacked position, other 64 rows are 0) so
            # the scores matmul streams a full-width 128-partition rhs.
            qT = qk_pool.tile([128, 3, S], bf16)
            kTz = qk_pool.tile([128, 3, 2, S], bf16)
            nc.gpsimd.memset(kTz, 0.0)
            for hp in range(3):
                for sh in range(QH):
                    ssl = slice(sh * 512, (sh + 1) * 512)
                    psq = proj_psum.tile([128, 512], f32, tag="proj")
                    for k in range(KC):
                        nc.tensor.matmul(psq, wq_sb[:, k, hp * 128:(hp + 1) * 128],
                                         hsT[:, k, ssl],
                                         start=(k == 0), stop=(k == KC - 1))
                    nc.scalar.activation(out=qT[:, hp, ssl], in_=psq, func=AF.Identity,
                                         bias=bq_sb[:, hp:hp + 1], scale=0.125)
                    psk = proj_psum.tile([128, 512], f32, tag="proj")
                    for k in range(KC):
                        nc.tensor.matmul(psk, wk_sb[:, k, hp * 128:(hp + 1) * 128],
                                         hsT[:, k, ssl],
                                         start=(k == 0), stop=(k == KC - 1))
                    nc.scalar.activation(out=kTz[0:64, hp, 0, ssl], in_=psk[0:64, :],
                                         func=AF.Identity,
                                         bias=bk_sb[0:64, hp:hp + 1], scale=1.0)
                    nc.scalar.activation(out=kTz[64:128, hp, 1, ssl],
                                         in_=psk[64:128, :], func=AF.Identity,
                                         bias=bk_sb[64:128, hp:hp + 1], scale=1.0)

            # v: [S, head, d] layout + ones column (col 64) for denominators
            v_sb = v_pool.tile([128, SC, HPC, HD + 1], bf16)
            nc.gpsimd.memset(v_sb[:, :, :, HD], 1.0)
            for sc in range(SC):
                psv_full = proj_psum.tile([128, 512], f32, tag="proj")
                psv = psv_full[:, 0:384]
                for k in range(KC):
                    nc.tensor.matmul(psv, hsT[:, k, sc * 128:(sc + 1) * 128],
                                     wv_sb[:, k, :],
                                     start=(k == 0), stop=(k == KC - 1))
                nc.vector.tensor_add(
                    v_sb[:, sc, :, 0:HD],
                    psv.rearrange("p (h d) -> p h d", h=HPC),
                    bv_sb.rearrange("p (h d) -> p h d", h=HPC))
            proj_ctx.__exit__(None, None, None)

            # ---- attention ----
            att_ctx = [tc.tile_pool(name="psS", bufs=4, space="PSUM"),
                       tc.tile_pool(name="psV", bufs=2, space="PSUM"),
                       tc.tile_pool(name="psT", bufs=2, space="PSUM")]
            sc_psum, pv_psum, pt_psum = [c.__enter__() for c in att_ctx]
            for h in range(HPC):
                hp, hi = divmod(h, 2)
                dsl = slice(hi * 64, (hi + 1) * 64)
                for qh in range(QH):
                    qsl = slice(qh * 512, (qh + 1) * 512)
                    r1 = r1_pool.tile([128, 4, S], bf16)
                    nc.sync.dma_start(
                        out=r1, in_=rel1[h, qh * 512:(qh + 1) * 512, :]
                        .rearrange("(i p) k -> p i k", p=128))
                    r2 = r2_pool.tile([128, 4, S], bf16)
                    nc.sync.dma_start(
                        out=r2, in_=rel2[h, qh * 512:(qh + 1) * 512, :]
                        .rearrange("(i p) k -> p i k", p=128))
                    r12 = r12_pool.tile([128, 4, S], f32r)
                    nc.vector.tensor_add(r12, r1, r2)

                    ets = []
                    for kc in range(SC):
                        ksl = slice(kc * 128, (kc + 1) * 128)
                        ps = sc_psum.tile([128, 512], f32)
                        # rel12[q,k].T accumulated into scoresT psum
                        for i in range(4):
                            nc.tensor.matmul(
                                ps[:, i * 128:(i + 1) * 128].bitcast(f32r),
                                r12[:, i, ksl], ident_r,
                                is_transpose=True, start=(i == 0), stop=False)
                        # += k^T q (zero-padded K=128: head hi's rows live at
                        # dsl, the other head's qT rows hit zeros in kTz)
                        nc.tensor.matmul(ps, kTz[:, hp, hi, ksl], qT[:, hp, qsl],
                                         start=False, stop=True)
                        et_kc = e_pool.tile([128, 512], bf16, tag="et")
                        ets.append(et_kc)
                        nc.scalar.activation(out=et_kc, in_=ps, func=AF.Exp,
                                             bias=maskb[:, kc:kc + 1], scale=1.0)

                    pv = pv_psum.tile([HD + 1, 512], f32)
                    for kc in range(SC):
                        nc.tensor.matmul(pv, v_sb[:, kc, h, :], ets[kc],
                                         start=(kc == 0), stop=(kc == SC - 1))
                    ctxT = ctxt_pool.tile([HD + 1, 512], f32)
                    nc.scalar.copy(ctxT, pv)
                    for i in range(4):
                        pt = pt_psum.tile([128, HD + 1], f32)
                        nc.tensor.matmul(pt, ctxT[:, i * 128:(i + 1) * 128],
                                         ident_f32[:HD + 1, :HD + 1],
                                         is_transpose=True, start=True, stop=True)
                        rec = ob_pool.tile([128, 1], f32, tag="rec")
                        nc.vector.reciprocal(rec, pt[:, HD:HD + 1])
                        ob = ob_pool.tile([128, HD], f32, tag="ob")
                        nc.vector.tensor_scalar_mul(ob, pt[:, 0:HD], rec)
                        nc.sync.dma_start(
                            out=out[qh * 512 + i * 128: qh * 512 + (i + 1) * 128,
                                    h * HD:(h + 1) * HD],
                            in_=ob)
            for c in reversed(att_ctx):
                c.__exit__(None, None, None)

    nc.compile()
    return nc


def _get_compiled():
    global _compiled
    if _compiled is None:
        _compiled = _build()
    return _compiled


def kernel(hidden_states, Wq, bq, Wk, bk, Wv, bv, rel_pos, rel_2d_pos,
           attention_mask, _trace=False):
    global last_result
    nc = _get_compiled()

    hidden_states = np.asarray(hidden_states, np.float32)
    Wq, Wk, Wv = (np.asarray(w, np.float32) for w in (Wq, Wk, Wv))
    bq, bk, bv = (np.asarray(x, np.float32) for x in (bq, bk, bv))
    rel_pos = np.asarray(rel_pos, np.float32)
    rel_2d_pos = np.asarray(rel_2d_pos, np.float32)
    attention_mask = np.asarray(attention_mask, np.int32)

    in_maps = []
    for c in range(N_CORES):
        b, hg = divmod(c, 2)
        cs = slice(hg * COLS, (hg + 1) * COLS)
        h0 = hg * HPC
        in_maps.append({
            "hs": hidden_states[b].astype(BF16_NP),
            "wq": Wq[:, cs].astype(BF16_NP),
            "wk": Wk[:, cs].astype(BF16_NP),
            "wv": Wv[:, cs].astype(BF16_NP),
            "bq": np.ascontiguousarray(bq[cs]) * np.float32(0.125),
            "bk": np.ascontiguousarray(bk[cs]),
            "bv": np.ascontiguousarray(bv[cs]),
            "rel1": rel_pos[b, h0:h0 + HPC].astype(BF16_NP),
            "rel2": rel_2d_pos[b, h0:h0 + HPC].astype(BF16_NP),
            "mask": np.ascontiguousarray(attention_mask[b, 0, 0]),
        })

    kwargs = {}
    if _trace or os.environ.get("KERNEL_TRACE"):
        kwargs["trace"] = True
    last_result = run_bass_kernel_spmd(nc, in_maps, list(range(N_CORES)), **kwargs)

    result = np.empty((B, S, H), np.float32)
    for c in range(N_CORES):
        b, hg = divmod(c, 2)
        result[b, :, hg * COLS:(hg + 1) * COLS] = last_result.results[c]["out"]
    return result
